# revision 3
# baseline (speedup 1.0000x reference)
"""DeltaNet (chunk-wise delta-rule linear attention) on 8 TRN2 NeuronCores.

Shapes: q,k,v [2,8,4096,128] f32, beta [2,8,4096] f32, chunk_size=32.
Returns (o [2,8,4096,128], S [2,8,128,128]) matching the reference.

Sharding: batch*heads = 16 independent (b,h) slices -> 2 per core.

Device algorithm (per slice, mathematically identical to the reference for
any chunk size; we use C=128 rows per chunk, verified to ~8e-7 rel err):
  qn,kn = l2norm(q,k); vb = v*beta; kb = kn*beta
  per chunk: A = strict_lower(-(kb kn^T));  M = (I-A)^{-1}
    split A = D (32-blockdiag) + E;  MD = (I-D)^{-1} via doubling on the
    transposed blockdiag (nilpotent at 32), then
    U = [u|w] = MD(X + E MD(X + E MD(X + E MD X))), X=[vb|kb]  (F^4=0)
  recurrence over chunks: u' = u - w S; o = q S + tril(q k^T) u'; S += kn^T u'
"""

import numpy as np

import concourse.bass as bass
import concourse.bacc as bacc
import concourse.tile as tile
import concourse.mybir as mybir
from concourse import bass_utils

N_CORES = 8
B, H, L, D = 2, 8, 4096, 128
NSL = B * H                      # 16 slices
SPC = NSL // N_CORES             # 2 slices per core
C = 128                          # device chunk rows
NCH = L // C                     # 32 chunks per slice
SUB = 32                         # sub-block for blockdiag inverse

f32 = mybir.dt.float32
AF = mybir.ActivationFunctionType


def _emit_chunk(nc, pools, consts, state, c, s):
    pio, pw, pp = pools
    ident, mdt, met, mal, eps, q_d, k_d, v_d, beta_d, o_d, s_d = consts
    S_ps, S_sb = state
    rows = slice(c * C, (c + 1) * C)

    # ---- A: load + preprocess (l2norm, beta scaling) ----
    qt = pio.tile([C, D], f32, tag="qin", name=f"qt{c}_{s}")
    kt = pio.tile([C, D], f32, tag="kin", name=f"kt{c}_{s}")
    vt = pio.tile([C, D], f32, tag="vin", name=f"vt{c}_{s}")
    bt = pio.tile([C, 1], f32, tag="bin", name=f"bt{c}_{s}")
    nc.sync.dma_start(qt[:], q_d[s, rows, :])
    nc.sync.dma_start(kt[:], k_d[s, rows, :])
    nc.sync.dma_start(vt[:], v_d[s, rows, :])
    nc.sync.dma_start(bt[:], beta_d[s, rows, :])

    scr = pw.tile([C, D], f32, tag="scr", name=f"scr{c}_{s}")
    ssq = pw.tile([C, 2], f32, tag="ssq", name=f"ssq{c}_{s}")
    nc.scalar.activation(scr[:], qt[:], AF.Square, accum_out=ssq[:, 0:1])
    nc.scalar.activation(scr[:], kt[:], AF.Square, accum_out=ssq[:, 1:2])
    sqs = pw.tile([C, 2], f32, tag="sqs", name=f"sqs{c}_{s}")
    nc.scalar.activation(sqs[:], ssq[:], AF.Sqrt, bias=eps[:])
    rqk = pw.tile([C, 2], f32, tag="rqk", name=f"rqk{c}_{s}")
    nc.vector.reciprocal(rqk[:], sqs[:])

    qn = pw.tile([C, D], f32, tag="qn", name=f"qn{c}_{s}")
    kn = pw.tile([C, D], f32, tag="kn", name=f"kn{c}_{s}")
    X = pw.tile([C, 2 * D], f32, tag="X", name=f"X{c}_{s}")
    nc.vector.tensor_scalar_mul(qn[:], qt[:], rqk[:, 0:1])
    nc.vector.tensor_scalar_mul(kn[:], kt[:], rqk[:, 1:2])
    nc.vector.tensor_scalar_mul(X[:, 0:D], vt[:], bt[:])        # vb
    nc.vector.tensor_scalar(X[:, D:], kt[:], rqk[:, 1:2], bt[:],
                            mybir.AluOpType.mult, mybir.AluOpType.mult)  # kb

    # ---- B: transposes + fused gram ----
    kTp = pp.tile([D, C], f32, tag="pp", name=f"kTp{c}_{s}")
    nc.tensor.transpose(kTp[:], kn[:], ident[:])
    kT = pw.tile([D, C], f32, tag="kT", name=f"kT{c}_{s}")
    nc.scalar.copy(kT[:], kTp[:])

    RT = pw.tile([D, 2 * C], f32, tag="RT", name=f"RT{c}_{s}")   # [kb^T | q^T]
    kbTp = pp.tile([D, C], f32, tag="pp", name=f"kbTp{c}_{s}")
    nc.tensor.transpose(kbTp[:], X[:, D:], ident[:])
    nc.scalar.copy(RT[:, 0:C], kbTp[:])
    qTp = pp.tile([D, C], f32, tag="pp", name=f"qTp{c}_{s}")
    nc.tensor.transpose(qTp[:], qn[:], ident[:])
    nc.scalar.copy(RT[:, C:], qTp[:])

    GKp = pp.tile([D, 2 * C], f32, tag="pp", name=f"GKp{c}_{s}")  # [G^T | KQ^T]
    nc.tensor.matmul(GKp[:], lhsT=kT[:], rhs=RT[:])

    DTs = pw.tile([D, C], f32, tag="DTs", name=f"DTs{c}_{s}")
    ETs = pw.tile([D, C], f32, tag="ETs", name=f"ETs{c}_{s}")
    ALTs = pw.tile([D, C], f32, tag="ALTs", name=f"ALTs{c}_{s}")
    nc.vector.tensor_mul(DTs[:], GKp[:, 0:C], mdt[:])   # mask holds -1/0
    nc.vector.tensor_mul(ETs[:], GKp[:, 0:C], met[:])   # mask holds -1/0
    nc.vector.tensor_mul(ALTs[:], GKp[:, C:], mal[:])   # mask holds 1/0

    # ---- C: MD^T = (I - D^T)^{-1} via doubling (blockdiag, nilpotent@32) ----
    DTtp = pp.tile([D, C], f32, tag="pp", name=f"DTtp{c}_{s}")
    nc.tensor.transpose(DTtp[:], DTs[:], ident[:])
    DTts = pw.tile([D, C], f32, tag="PTs", name=f"DTts{c}_{s}")
    nc.scalar.copy(DTts[:], DTtp[:])
    P2p = pp.tile([D, C], f32, tag="pp", name=f"P2p{c}_{s}")
    nc.tensor.matmul(P2p[:], lhsT=DTts[:], rhs=DTs[:])           # DT@DT

    PM = pw.tile([D, 2 * C], f32, tag="PM", name=f"PM{c}_{s}")   # [P_k | MT_k]
    nc.scalar.copy(PM[:, 0:C], P2p[:])
    nc.vector.tensor_add(PM[:, C:], DTs[:], ident[:])            # MT_2 = I+DT

    for lvl in range(3):                                         # k = 2,4,8
        PTp = pp.tile([D, C], f32, tag="pp", name=f"PTp{c}_{s}_{lvl}")
        nc.tensor.transpose(PTp[:], PM[:, 0:C], ident[:])
        PTs = pw.tile([D, C], f32, tag="PTs", name=f"PTs{c}_{s}_{lvl}")
        nc.scalar.copy(PTs[:], PTp[:])
        Bp = pp.tile([D, 2 * C], f32, tag="pp", name=f"Bp{c}_{s}_{lvl}")
        nc.tensor.matmul(Bp[:], lhsT=PTs[:], rhs=PM[:])          # [P@P | P@MT]
        PMn = pw.tile([D, 2 * C], f32, tag="PM", name=f"PM{c}_{s}_{lvl}")
        nc.scalar.copy(PMn[:, 0:C], Bp[:, 0:C])
        nc.vector.tensor_add(PMn[:, C:], PM[:, C:], Bp[:, C:])
        PM = PMn

    PTp = pp.tile([D, C], f32, tag="pp", name=f"PTpf{c}_{s}")    # k=16: MT only
    nc.tensor.transpose(PTp[:], PM[:, 0:C], ident[:])
    PTs = pw.tile([D, C], f32, tag="PTs", name=f"PTsf{c}_{s}")
    nc.scalar.copy(PTs[:], PTp[:])
    Cp = pp.tile([D, C], f32, tag="pp", name=f"Cp{c}_{s}")
    nc.tensor.matmul(Cp[:], lhsT=PTs[:], rhs=PM[:, C:])
    MT = pw.tile([D, C], f32, tag="MT", name=f"MT{c}_{s}")
    nc.vector.tensor_add(MT[:], PM[:, C:], Cp[:])

    # ---- D: solve U = MD(X + E MD(X + E MD(X + E MD X))) ----
    cur = X
    for it in range(3):
        Tp = pp.tile([D, 2 * C], f32, tag="pp", name=f"Tp{c}_{s}_{it}")
        nc.tensor.matmul(Tp[:], lhsT=MT[:], rhs=cur[:])
        Ts = pw.tile([D, 2 * C], f32, tag="Ts", name=f"Ts{c}_{s}_{it}")
        nc.scalar.copy(Ts[:], Tp[:])
        Ep = pp.tile([D, 2 * C], f32, tag="pp", name=f"Ep{c}_{s}_{it}")
        nc.tensor.matmul(Ep[:], lhsT=ETs[:], rhs=Ts[:])
        Xn = pw.tile([D, 2 * C], f32, tag="Xn", name=f"Xn{c}_{s}_{it}")
        nc.vector.tensor_add(Xn[:], X[:], Ep[:])
        cur = Xn
    Up = pp.tile([D, 2 * C], f32, tag="pp", name=f"Up{c}_{s}")
    nc.tensor.matmul(Up[:], lhsT=MT[:], rhs=cur[:])
    U = pw.tile([D, 2 * C], f32, tag="U", name=f"U{c}_{s}")
    nc.scalar.copy(U[:], Up[:])

    # ---- E: inter-chunk recurrence ----
    WTp = pp.tile([D, C], f32, tag="pp", name=f"WTp{c}_{s}")
    nc.tensor.transpose(WTp[:], U[:, D:], ident[:])
    WTs = pw.tile([D, C], f32, tag="WTs", name=f"WTs{c}_{s}")
    nc.scalar.copy(WTs[:], WTp[:])

    if c == 0:
        up_ap = U[:, 0:D]
    else:
        upp = pp.tile([C, D], f32, tag="pp", name=f"upp{c}_{s}")
        nc.tensor.matmul(upp[:], lhsT=WTs[:], rhs=S_sb[s][:])
        ups = pw.tile([C, D], f32, tag="ups", name=f"ups{c}_{s}")
        nc.vector.tensor_sub(ups[:], U[:, 0:D], upp[:])
        up_ap = ups

    op_ = pp.tile([C, D], f32, tag="pp", name=f"op{c}_{s}")
    qT = RT[:, C:]
    if c == 0:
        nc.tensor.matmul(op_[:], lhsT=ALTs[:], rhs=up_ap[:])
    else:
        nc.tensor.matmul(op_[:], lhsT=qT, rhs=S_sb[s][:], start=True, stop=False)
        nc.tensor.matmul(op_[:], lhsT=ALTs[:], rhs=up_ap[:], start=False, stop=True)
    os_ = pw.tile([C, D], f32, tag="os", name=f"os{c}_{s}")
    nc.scalar.copy(os_[:], op_[:])
    nc.sync.dma_start(o_d[s, rows, :], os_[:])

    nc.tensor.matmul(S_ps[s][:], lhsT=kn[:], rhs=up_ap[:],
                     start=(c == 0), stop=(c == NCH - 1), skip_group_check=True)
    Ssb = pw.tile([D, D], f32, tag=f"Ssb{s}", name=f"Ssb{c}_{s}")
    nc.scalar.copy(Ssb[:], S_ps[s][:])
    if c == NCH - 1:
        nc.sync.dma_start(s_d[s], Ssb[:])
    else:
        S_sb[s] = Ssb


def build(nch=NCH):
    global NCH
    NCH = nch
    ll = nch * C
    nc = bacc.Bacc("TRN2", target_bir_lowering=False, debug=False,
                   num_devices=N_CORES)
    q_d = nc.dram_tensor("q", (SPC, ll, D), f32, kind="ExternalInput").ap()
    k_d = nc.dram_tensor("k", (SPC, ll, D), f32, kind="ExternalInput").ap()
    v_d = nc.dram_tensor("v", (SPC, ll, D), f32, kind="ExternalInput").ap()
    beta_d = nc.dram_tensor("beta", (SPC, ll, 1), f32, kind="ExternalInput").ap()
    ident_d = nc.dram_tensor("ident", (D, D), f32, kind="ExternalInput").ap()
    mdt_d = nc.dram_tensor("mdt", (D, D), f32, kind="ExternalInput").ap()
    met_d = nc.dram_tensor("met", (D, D), f32, kind="ExternalInput").ap()
    mal_d = nc.dram_tensor("mal", (D, D), f32, kind="ExternalInput").ap()
    o_d = nc.dram_tensor("o", (SPC, ll, D), f32, kind="ExternalOutput").ap()
    s_d = nc.dram_tensor("s_out", (SPC, D, D), f32, kind="ExternalOutput").ap()

    with tile.TileContext(nc) as tc:
        with tc.tile_pool(name="consts", bufs=1) as pc, \
             tc.tile_pool(name="io", bufs=6) as pio, \
             tc.tile_pool(name="work", bufs=3) as pw, \
             tc.tile_pool(name="pp", bufs=6, space="PSUM") as pp, \
             tc.tile_pool(name="pS", bufs=1, space="PSUM") as pS:
            ident = pc.tile([D, D], f32, name="ident")
            mdt = pc.tile([D, D], f32, name="mdt")
            met = pc.tile([D, D], f32, name="met")
            mal = pc.tile([D, D], f32, name="mal")
            eps = pc.tile([D, 1], f32, name="eps")
            nc.vector.memset(eps[:], 1e-6)
            nc.sync.dma_start(ident[:], ident_d[:])
            nc.sync.dma_start(mdt[:], mdt_d[:])
            nc.sync.dma_start(met[:], met_d[:])
            nc.sync.dma_start(mal[:], mal_d[:])

            S_ps = [pS.tile([D, D], f32, name=f"Sps{s}") for s in range(SPC)]
            S_sb = [None] * SPC
            pools = (pio, pw, pp)
            consts = (ident, mdt, met, mal, eps, q_d, k_d, v_d, beta_d, o_d, s_d)
            state = (S_ps, S_sb)
            for c in range(nch):
                for s in range(SPC):
                    _emit_chunk(nc, pools, consts, state, c, s)
    nc.compile()
    return nc


def _consts():
    ii, jj = np.meshgrid(np.arange(C), np.arange(C), indexing="ij")
    bi, bj = ii // SUB, jj // SUB
    mdt = np.where((bi == bj) & (jj > ii), -1.0, 0.0).astype(np.float32)
    met = np.where(bj > bi, -1.0, 0.0).astype(np.float32)
    mal = np.where(jj >= ii, 1.0, 0.0).astype(np.float32)
    ident = np.eye(C, dtype=np.float32)
    return ident, mdt, met, mal


_NC_CACHE = {}


def kernel(q, k, v, beta, chunk_size=None):
    q = np.ascontiguousarray(np.asarray(q, dtype=np.float32))
    k = np.ascontiguousarray(np.asarray(k, dtype=np.float32))
    v = np.ascontiguousarray(np.asarray(v, dtype=np.float32))
    beta = np.ascontiguousarray(np.asarray(beta, dtype=np.float32))
    b, h, ll, d = q.shape
    assert (b, h, ll, d) == (B, H, L, D)

    if "nc" not in _NC_CACHE:
        _NC_CACHE["nc"] = build()
    nc = _NC_CACHE["nc"]

    ident, mdt, met, mal = _consts()
    qf = q.reshape(NSL, ll, d)
    kf = k.reshape(NSL, ll, d)
    vf = v.reshape(NSL, ll, d)
    bf = beta.reshape(NSL, ll, 1)
    in_maps = []
    for i in range(N_CORES):
        sl = slice(i * SPC, (i + 1) * SPC)
        in_maps.append({
            "q": qf[sl], "k": kf[sl], "v": vf[sl], "beta": bf[sl],
            "ident": ident, "mdt": mdt, "met": met, "mal": mal,
        })
    res = bass_utils.run_bass_kernel_spmd(nc, in_maps, core_ids=list(range(N_CORES)))
    o = np.concatenate([res.results[i]["o"] for i in range(N_CORES)], axis=0)
    S = np.concatenate([res.results[i]["s_out"] for i in range(N_CORES)], axis=0)
    return o.reshape(B, H, ll, d), S.reshape(B, H, d, d)


# revision 4
# speedup vs baseline: 1.4204x; 1.4204x over previous
"""DeltaNet (chunk-wise delta-rule linear attention) on 8 TRN2 NeuronCores.

Shapes: q,k,v [2,8,4096,128] f32, beta [2,8,4096] f32, chunk_size=32.
Returns (o [2,8,4096,128], S [2,8,128,128]) matching the reference.

Sharding: batch*heads = 16 independent (b,h) slices -> 2 per core.

Device algorithm (per slice, mathematically identical to the reference for
any chunk size; we use C=128 rows per chunk, verified to ~8e-7 rel err):
  qn,kn = l2norm(q,k); vb = v*beta; kb = kn*beta
  per chunk: A = strict_lower(-(kb kn^T));  M = (I-A)^{-1}
    split A = D (32-blockdiag) + E;  MD = (I-D)^{-1} via doubling on the
    transposed blockdiag (nilpotent at 32), then
    U = [u|w] = MD(X + E MD(X + E MD(X + E MD X))), X=[vb|kb]  (F^4=0)
  recurrence over chunks: u' = u - w S; o = q S + tril(q k^T) u'; S += kn^T u'
"""

import numpy as np

import concourse.bass as bass
import concourse.bacc as bacc
import concourse.tile as tile
import concourse.mybir as mybir
from concourse import bass_utils

N_CORES = 8
B, H, L, D = 2, 8, 4096, 128
NSL = B * H                      # 16 slices
SPC = NSL // N_CORES             # 2 slices per core
C = 128                          # device chunk rows
NCH = L // C                     # 32 chunks per slice
SUB = 32                         # sub-block for blockdiag inverse

f32 = mybir.dt.float32
fr = mybir.dt.float32r
AF = mybir.ActivationFunctionType


def _emit_chunk(nc, pools, consts, state, c, s):
    pio, pw, pp = pools
    ident, mdt, met, mal, eps, q_d, k_d, v_d, beta_d, o_d, s_d = consts
    S_ps, S_sb = state
    rows = slice(c * C, (c + 1) * C)

    # ---- A: load + preprocess (l2norm, beta scaling) ----
    qt = pio.tile([C, D], f32, tag="qin", name=f"qt{c}_{s}")
    kt = pio.tile([C, D], f32, tag="kin", name=f"kt{c}_{s}")
    vt = pio.tile([C, D], f32, tag="vin", name=f"vt{c}_{s}")
    bt = pio.tile([C, 1], f32, tag="bin", name=f"bt{c}_{s}")
    nc.sync.dma_start(qt[:], q_d[s, rows, :])
    nc.sync.dma_start(kt[:], k_d[s, rows, :])
    nc.sync.dma_start(vt[:], v_d[s, rows, :])
    nc.sync.dma_start(bt[:], beta_d[s, rows, :])

    scr = pw.tile([C, D], f32, tag="scr", name=f"scr{c}_{s}")
    ssq = pw.tile([C, 2], f32, tag="ssq", name=f"ssq{c}_{s}")
    nc.scalar.activation(scr[:], qt[:], AF.Square, accum_out=ssq[:, 0:1])
    nc.scalar.activation(scr[:], kt[:], AF.Square, accum_out=ssq[:, 1:2])
    sqs = pw.tile([C, 2], f32, tag="sqs", name=f"sqs{c}_{s}")
    nc.scalar.activation(sqs[:], ssq[:], AF.Sqrt, bias=eps[:])
    rqk = pw.tile([C, 2], f32, tag="rqk", name=f"rqk{c}_{s}")
    nc.vector.reciprocal(rqk[:], sqs[:])

    qn = pw.tile([C, D], fr, tag="qn", name=f"qn{c}_{s}")
    kn = pw.tile([C, D], fr, tag="kn", name=f"kn{c}_{s}")
    X = pw.tile([C, 2 * D], fr, tag="X", name=f"X{c}_{s}")
    nc.vector.tensor_scalar_mul(qn[:], qt[:], rqk[:, 0:1])
    nc.vector.tensor_scalar_mul(kn[:], kt[:], rqk[:, 1:2])
    nc.vector.tensor_scalar_mul(X[:, 0:D], vt[:], bt[:])        # vb
    nc.vector.tensor_scalar(X[:, D:], kt[:], rqk[:, 1:2], bt[:],
                            mybir.AluOpType.mult, mybir.AluOpType.mult)  # kb

    # ---- B: transposes + fused gram ----
    kTp = pp.tile([D, C], fr, tag="pp", name=f"kTp{c}_{s}")
    nc.tensor.transpose(kTp[:], kn[:], ident[:])
    kT = pw.tile([D, C], fr, tag="kT", name=f"kT{c}_{s}")
    nc.scalar.copy(kT[:], kTp[:])

    RT = pw.tile([D, 2 * C], fr, tag="RT", name=f"RT{c}_{s}")   # [kb^T | q^T]
    kbTp = pp.tile([D, C], fr, tag="pp", name=f"kbTp{c}_{s}")
    nc.tensor.transpose(kbTp[:], X[:, D:], ident[:])
    nc.scalar.copy(RT[:, 0:C], kbTp[:])
    qTp = pp.tile([D, C], fr, tag="pp", name=f"qTp{c}_{s}")
    nc.tensor.transpose(qTp[:], qn[:], ident[:])
    nc.scalar.copy(RT[:, C:], qTp[:])

    GKp = pp.tile([D, 2 * C], f32, tag="pp", name=f"GKp{c}_{s}")  # [G^T | KQ^T]
    nc.tensor.matmul(GKp[:], lhsT=kT[:], rhs=RT[:])

    DTs = pw.tile([D, C], fr, tag="DTs", name=f"DTs{c}_{s}")
    ETs = pw.tile([D, C], fr, tag="ETs", name=f"ETs{c}_{s}")
    ALTs = pw.tile([D, C], fr, tag="ALTs", name=f"ALTs{c}_{s}")
    nc.vector.tensor_mul(DTs[:], GKp[:, 0:C], mdt[:])   # mask holds -1/0
    nc.vector.tensor_mul(ETs[:], GKp[:, 0:C], met[:])   # mask holds -1/0
    nc.vector.tensor_mul(ALTs[:], GKp[:, C:], mal[:])   # mask holds 1/0

    # ---- C: MD^T = (I - D^T)^{-1} via doubling (blockdiag, nilpotent@32) ----
    DTtp = pp.tile([D, C], fr, tag="pp", name=f"DTtp{c}_{s}")
    nc.tensor.transpose(DTtp[:], DTs[:], ident[:])
    DTts = pw.tile([D, C], fr, tag="PTs", name=f"DTts{c}_{s}")
    nc.scalar.copy(DTts[:], DTtp[:])
    P2p = pp.tile([D, C], f32, tag="pp", name=f"P2p{c}_{s}")
    nc.tensor.matmul(P2p[:], lhsT=DTts[:], rhs=DTs[:])           # DT@DT

    PM = pw.tile([D, 2 * C], fr, tag="PM", name=f"PM{c}_{s}")   # [P_k | MT_k]
    nc.scalar.copy(PM[:, 0:C], P2p[:])
    nc.vector.tensor_add(PM[:, C:], DTs[:], ident[:])            # MT_2 = I+DT

    for lvl in range(3):                                         # k = 2,4,8
        PTp = pp.tile([D, C], fr, tag="pp", name=f"PTp{c}_{s}_{lvl}")
        nc.tensor.transpose(PTp[:], PM[:, 0:C], ident[:])
        PTs = pw.tile([D, C], fr, tag="PTs", name=f"PTs{c}_{s}_{lvl}")
        nc.scalar.copy(PTs[:], PTp[:])
        Bp = pp.tile([D, 2 * C], f32, tag="pp", name=f"Bp{c}_{s}_{lvl}")
        nc.tensor.matmul(Bp[:], lhsT=PTs[:], rhs=PM[:])          # [P@P | P@MT]
        PMn = pw.tile([D, 2 * C], fr, tag="PM", name=f"PM{c}_{s}_{lvl}")
        nc.scalar.copy(PMn[:, 0:C], Bp[:, 0:C])
        nc.vector.tensor_add(PMn[:, C:], PM[:, C:], Bp[:, C:])
        PM = PMn

    PTp = pp.tile([D, C], fr, tag="pp", name=f"PTpf{c}_{s}")    # k=16: MT only
    nc.tensor.transpose(PTp[:], PM[:, 0:C], ident[:])
    PTs = pw.tile([D, C], fr, tag="PTs", name=f"PTsf{c}_{s}")
    nc.scalar.copy(PTs[:], PTp[:])
    Cp = pp.tile([D, C], f32, tag="pp", name=f"Cp{c}_{s}")
    nc.tensor.matmul(Cp[:], lhsT=PTs[:], rhs=PM[:, C:])
    MT = pw.tile([D, C], fr, tag="MT", name=f"MT{c}_{s}")
    nc.vector.tensor_add(MT[:], PM[:, C:], Cp[:])

    # ---- D: solve U = MD(X + E MD(X + E MD(X + E MD X))) ----
    cur = X
    for it in range(3):
        Tp = pp.tile([D, 2 * C], f32, tag="pp", name=f"Tp{c}_{s}_{it}")
        nc.tensor.matmul(Tp[:], lhsT=MT[:], rhs=cur[:])
        Ts = pw.tile([D, 2 * C], fr, tag="Ts", name=f"Ts{c}_{s}_{it}")
        nc.scalar.copy(Ts[:], Tp[:])
        Ep = pp.tile([D, 2 * C], f32, tag="pp", name=f"Ep{c}_{s}_{it}")
        nc.tensor.matmul(Ep[:], lhsT=ETs[:], rhs=Ts[:])
        Xn = pw.tile([D, 2 * C], fr, tag="Xn", name=f"Xn{c}_{s}_{it}")
        nc.vector.tensor_add(Xn[:], X[:], Ep[:])
        cur = Xn
    Up = pp.tile([D, 2 * C], f32, tag="pp", name=f"Up{c}_{s}")
    nc.tensor.matmul(Up[:], lhsT=MT[:], rhs=cur[:])
    U = pw.tile([D, 2 * C], fr, tag="U", name=f"U{c}_{s}")
    nc.scalar.copy(U[:], Up[:])

    # ---- E: inter-chunk recurrence ----
    WTp = pp.tile([D, C], fr, tag="pp", name=f"WTp{c}_{s}")
    nc.tensor.transpose(WTp[:], U[:, D:], ident[:])
    WTs = pw.tile([D, C], fr, tag="WTs", name=f"WTs{c}_{s}")
    nc.scalar.copy(WTs[:], WTp[:])

    if c == 0:
        up_ap = U[:, 0:D]
    else:
        upp = pp.tile([C, D], f32, tag="pp", name=f"upp{c}_{s}")
        nc.tensor.matmul(upp[:], lhsT=WTs[:], rhs=S_sb[s][:])
        ups = pw.tile([C, D], fr, tag="ups", name=f"ups{c}_{s}")
        nc.vector.tensor_sub(ups[:], U[:, 0:D], upp[:])
        up_ap = ups

    op_ = pp.tile([C, D], f32, tag="pp", name=f"op{c}_{s}")
    qT = RT[:, C:]
    if c == 0:
        nc.tensor.matmul(op_[:], lhsT=ALTs[:], rhs=up_ap[:])
    else:
        nc.tensor.matmul(op_[:], lhsT=qT, rhs=S_sb[s][:], start=True, stop=False)
        nc.tensor.matmul(op_[:], lhsT=ALTs[:], rhs=up_ap[:], start=False, stop=True)
    os_ = pw.tile([C, D], f32, tag="os", name=f"os{c}_{s}")
    nc.scalar.copy(os_[:], op_[:])
    nc.sync.dma_start(o_d[s, rows, :], os_[:])

    nc.tensor.matmul(S_ps[s][:], lhsT=kn[:], rhs=up_ap[:],
                     start=(c == 0), stop=(c == NCH - 1), skip_group_check=True)
    Ssb = pw.tile([D, D], fr, tag=f"Ssb{s}", name=f"Ssb{c}_{s}")
    nc.scalar.copy(Ssb[:], S_ps[s][:])
    if c == NCH - 1:
        nc.sync.dma_start(s_d[s], Ssb[:].bitcast(f32))
    else:
        S_sb[s] = Ssb


def build(nch=NCH):
    global NCH
    NCH = nch
    ll = nch * C
    nc = bacc.Bacc("TRN2", target_bir_lowering=False, debug=False,
                   num_devices=N_CORES)
    q_d = nc.dram_tensor("q", (SPC, ll, D), f32, kind="ExternalInput").ap()
    k_d = nc.dram_tensor("k", (SPC, ll, D), f32, kind="ExternalInput").ap()
    v_d = nc.dram_tensor("v", (SPC, ll, D), f32, kind="ExternalInput").ap()
    beta_d = nc.dram_tensor("beta", (SPC, ll, 1), f32, kind="ExternalInput").ap()
    ident_d = nc.dram_tensor("ident", (D, D), fr, kind="ExternalInput").ap()
    mdt_d = nc.dram_tensor("mdt", (D, D), f32, kind="ExternalInput").ap()
    met_d = nc.dram_tensor("met", (D, D), f32, kind="ExternalInput").ap()
    mal_d = nc.dram_tensor("mal", (D, D), f32, kind="ExternalInput").ap()
    o_d = nc.dram_tensor("o", (SPC, ll, D), f32, kind="ExternalOutput").ap()
    s_d = nc.dram_tensor("s_out", (SPC, D, D), f32, kind="ExternalOutput").ap()

    with tile.TileContext(nc) as tc:
        with tc.tile_pool(name="consts", bufs=1) as pc, \
             tc.tile_pool(name="io", bufs=6) as pio, \
             tc.tile_pool(name="work", bufs=3) as pw, \
             tc.tile_pool(name="pp", bufs=6, space="PSUM") as pp, \
             tc.tile_pool(name="pS", bufs=1, space="PSUM") as pS:
            ident = pc.tile([D, D], fr, name="ident")
            mdt = pc.tile([D, D], f32, name="mdt")
            met = pc.tile([D, D], f32, name="met")
            mal = pc.tile([D, D], f32, name="mal")
            eps = pc.tile([D, 1], f32, name="eps")
            nc.vector.memset(eps[:], 1e-6)
            nc.sync.dma_start(ident[:], ident_d[:])
            nc.sync.dma_start(mdt[:], mdt_d[:])
            nc.sync.dma_start(met[:], met_d[:])
            nc.sync.dma_start(mal[:], mal_d[:])

            S_ps = [pS.tile([D, D], f32, name=f"Sps{s}") for s in range(SPC)]
            S_sb = [None] * SPC
            pools = (pio, pw, pp)
            consts = (ident, mdt, met, mal, eps, q_d, k_d, v_d, beta_d, o_d, s_d)
            state = (S_ps, S_sb)
            for c in range(nch):
                for s in range(SPC):
                    _emit_chunk(nc, pools, consts, state, c, s)
    nc.compile()
    return nc


def _consts():
    ii, jj = np.meshgrid(np.arange(C), np.arange(C), indexing="ij")
    bi, bj = ii // SUB, jj // SUB
    mdt = np.where((bi == bj) & (jj > ii), -1.0, 0.0).astype(np.float32)
    met = np.where(bj > bi, -1.0, 0.0).astype(np.float32)
    mal = np.where(jj >= ii, 1.0, 0.0).astype(np.float32)
    ident = np.eye(C, dtype=np.float32)
    return ident, mdt, met, mal


_NC_CACHE = {}


def kernel(q, k, v, beta, chunk_size=None):
    q = np.ascontiguousarray(np.asarray(q, dtype=np.float32))
    k = np.ascontiguousarray(np.asarray(k, dtype=np.float32))
    v = np.ascontiguousarray(np.asarray(v, dtype=np.float32))
    beta = np.ascontiguousarray(np.asarray(beta, dtype=np.float32))
    b, h, ll, d = q.shape
    assert (b, h, ll, d) == (B, H, L, D)

    if "nc" not in _NC_CACHE:
        _NC_CACHE["nc"] = build()
    nc = _NC_CACHE["nc"]

    ident, mdt, met, mal = _consts()
    qf = q.reshape(NSL, ll, d)
    kf = k.reshape(NSL, ll, d)
    vf = v.reshape(NSL, ll, d)
    bf = beta.reshape(NSL, ll, 1)
    in_maps = []
    for i in range(N_CORES):
        sl = slice(i * SPC, (i + 1) * SPC)
        in_maps.append({
            "q": qf[sl], "k": kf[sl], "v": vf[sl], "beta": bf[sl],
            "ident": ident, "mdt": mdt, "met": met, "mal": mal,
        })
    res = bass_utils.run_bass_kernel_spmd(nc, in_maps, core_ids=list(range(N_CORES)))
    o = np.concatenate([res.results[i]["o"] for i in range(N_CORES)], axis=0)
    S = np.concatenate([res.results[i]["s_out"] for i in range(N_CORES)], axis=0)
    return o.reshape(B, H, ll, d), S.reshape(B, H, d, d)


# revision 5
# speedup vs baseline: 1.4671x; 1.0329x over previous
"""DeltaNet (chunk-wise delta-rule linear attention) on 8 TRN2 NeuronCores.

Shapes: q,k,v [2,8,4096,128] f32, beta [2,8,4096] f32, chunk_size=32.
Returns (o [2,8,4096,128], S [2,8,128,128]) matching the reference.

Sharding: batch*heads = 16 independent (b,h) slices -> 2 per core.

Device algorithm (per slice, mathematically identical to the reference for
any chunk size; C=128 rows per device chunk):
  qn,kn = l2norm(q,k); vb = v*beta; kb = kn*beta
  per chunk: A = strict_lower(-(kb kn^T));  M = (I-A)^{-1}
    split A = D (32-blockdiag) + E;  MD = (I-D)^{-1} via doubling on the
    transposed blockdiag (nilpotent at 32), then
    U = [u|w] = MD(X + E MD(X + E MD(X + E MD X))), X=[vb|kb]  (F^4=0)
  recurrence over chunks: u' = u - w S; o = q S + tril(q k^T) u'; S += kn^T u'

Matmul operands are fp16 (PSUM accumulation stays fp32); transposes are
regular matmuls against an fp16 identity so weight loads pipeline.
"""

import numpy as np

import concourse.bass as bass
import concourse.bacc as bacc
import concourse.tile as tile
import concourse.mybir as mybir
from concourse import bass_utils

N_CORES = 8
B, H, L, D = 2, 8, 4096, 128
NSL = B * H                      # 16 slices
SPC = NSL // N_CORES             # 2 slices per core
C = 128                          # device chunk rows
NCH = L // C                     # 32 chunks per slice
SUB = 32                         # sub-block for blockdiag inverse

f32 = mybir.dt.float32
f16 = mybir.dt.float16
AF = mybir.ActivationFunctionType
MUL = mybir.AluOpType.mult


def _emit_chunk(nc, pools, consts, state, c, s):
    pio, pw, pp = pools
    ident, mdt, met, mal, eps, q_d, k_d, v_d, beta_d, o_d, s_d = consts
    S_ps, S_sb = state
    rows = slice(c * C, (c + 1) * C)

    # ---- A: load + preprocess (l2norm, beta scaling) ----
    qt = pio.tile([C, D], f32, tag="qin", name=f"qt{c}_{s}")
    kt = pio.tile([C, D], f32, tag="kin", name=f"kt{c}_{s}")
    vt = pio.tile([C, D], f32, tag="vin", name=f"vt{c}_{s}")
    bt = pio.tile([C, 1], f32, tag="bin", name=f"bt{c}_{s}")
    nc.sync.dma_start(qt[:], q_d[s, rows, :])
    nc.sync.dma_start(kt[:], k_d[s, rows, :])
    nc.sync.dma_start(vt[:], v_d[s, rows, :])
    nc.sync.dma_start(bt[:], beta_d[s, rows, :])

    scr = pw.tile([C, D], f32, tag="scr", name=f"scr{c}_{s}")
    ssq = pw.tile([C, 2], f32, tag="ssq", name=f"ssq{c}_{s}")
    nc.scalar.activation(scr[:], qt[:], AF.Square, accum_out=ssq[:, 0:1])
    nc.scalar.activation(scr[:], kt[:], AF.Square, accum_out=ssq[:, 1:2])
    sqs = pw.tile([C, 2], f32, tag="sqs", name=f"sqs{c}_{s}")
    nc.scalar.activation(sqs[:], ssq[:], AF.Sqrt, bias=eps[:])
    rqk = pw.tile([C, 2], f32, tag="rqk", name=f"rqk{c}_{s}")
    nc.vector.reciprocal(rqk[:], sqs[:])

    qn = pw.tile([C, D], f16, tag="qn", name=f"qn{c}_{s}")
    kn = pw.tile([C, D], f16, tag="kn", name=f"kn{c}_{s}")
    X = pw.tile([C, 2 * D], f16, tag="X", name=f"X{c}_{s}")
    nc.gpsimd.tensor_scalar_mul(qn[:], qt[:], rqk[:, 0:1])
    nc.gpsimd.tensor_scalar_mul(kn[:], kt[:], rqk[:, 1:2])
    nc.gpsimd.tensor_scalar_mul(X[:, 0:D], vt[:], bt[:])        # vb
    nc.gpsimd.tensor_scalar(X[:, D:], kt[:], rqk[:, 1:2], bt[:], MUL, MUL)  # kb

    # ---- B: transposes (as plain matmuls vs identity) + fused gram ----
    kTp = pp.tile([D, C], f32, tag="pp", name=f"kTp{c}_{s}")
    nc.tensor.matmul(kTp[:], lhsT=kn[:], rhs=ident[:])
    kT = pw.tile([D, C], f16, tag="kT", name=f"kT{c}_{s}")
    nc.scalar.copy(kT[:], kTp[:])

    RT = pw.tile([D, 2 * C], f16, tag="RT", name=f"RT{c}_{s}")   # [kb^T | q^T]
    kbTp = pp.tile([D, C], f32, tag="pp", name=f"kbTp{c}_{s}")
    nc.tensor.matmul(kbTp[:], lhsT=X[:, D:], rhs=ident[:])
    nc.scalar.copy(RT[:, 0:C], kbTp[:])
    qTp = pp.tile([D, C], f32, tag="pp", name=f"qTp{c}_{s}")
    nc.tensor.matmul(qTp[:], lhsT=qn[:], rhs=ident[:])
    nc.scalar.copy(RT[:, C:], qTp[:])

    GKp = pp.tile([D, 2 * C], f32, tag="pp", name=f"GKp{c}_{s}")  # [G^T | KQ^T]
    nc.tensor.matmul(GKp[:], lhsT=kT[:], rhs=RT[:])

    DTs = pw.tile([D, C], f16, tag="DTs", name=f"DTs{c}_{s}")
    ETs = pw.tile([D, C], f16, tag="ETs", name=f"ETs{c}_{s}")
    ALTs = pw.tile([D, C], f16, tag="ALTs", name=f"ALTs{c}_{s}")
    nc.vector.tensor_mul(DTs[:], GKp[:, 0:C], mdt[:])   # mask holds -1/0
    nc.vector.tensor_mul(ETs[:], GKp[:, 0:C], met[:])   # mask holds -1/0
    nc.vector.tensor_mul(ALTs[:], GKp[:, C:], mal[:])   # mask holds 1/0

    # ---- C: MD^T = (I - D^T)^{-1} via doubling (blockdiag, nilpotent@32) ----
    DTtp = pp.tile([D, C], f32, tag="pp", name=f"DTtp{c}_{s}")
    nc.tensor.matmul(DTtp[:], lhsT=DTs[:], rhs=ident[:])
    DTts = pw.tile([D, C], f16, tag="PTs", name=f"DTts{c}_{s}")
    nc.scalar.copy(DTts[:], DTtp[:])
    P2p = pp.tile([D, C], f32, tag="pp", name=f"P2p{c}_{s}")
    nc.tensor.matmul(P2p[:], lhsT=DTts[:], rhs=DTs[:])           # DT@DT

    PM = pw.tile([D, 2 * C], f16, tag="PM", name=f"PM{c}_{s}")   # [P_k | MT_k]
    nc.scalar.copy(PM[:, 0:C], P2p[:])
    nc.vector.tensor_add(PM[:, C:], DTs[:], ident[:])            # MT_2 = I+DT

    for lvl in range(3):                                         # k = 2,4,8
        PTp = pp.tile([D, C], f32, tag="pp", name=f"PTp{c}_{s}_{lvl}")
        nc.tensor.matmul(PTp[:], lhsT=PM[:, 0:C], rhs=ident[:])
        PTs = pw.tile([D, C], f16, tag="PTs", name=f"PTs{c}_{s}_{lvl}")
        nc.scalar.copy(PTs[:], PTp[:])
        Bp = pp.tile([D, 2 * C], f32, tag="pp", name=f"Bp{c}_{s}_{lvl}")
        nc.tensor.matmul(Bp[:], lhsT=PTs[:], rhs=PM[:])          # [P@P | P@MT]
        PMn = pw.tile([D, 2 * C], f16, tag="PM", name=f"PM{c}_{s}_{lvl}")
        nc.scalar.copy(PMn[:, 0:C], Bp[:, 0:C])
        nc.vector.tensor_add(PMn[:, C:], PM[:, C:], Bp[:, C:])
        PM = PMn

    PTp = pp.tile([D, C], f32, tag="pp", name=f"PTpf{c}_{s}")    # k=16: MT only
    nc.tensor.matmul(PTp[:], lhsT=PM[:, 0:C], rhs=ident[:])
    PTs = pw.tile([D, C], f16, tag="PTs", name=f"PTsf{c}_{s}")
    nc.scalar.copy(PTs[:], PTp[:])
    Cp = pp.tile([D, C], f32, tag="pp", name=f"Cp{c}_{s}")
    nc.tensor.matmul(Cp[:], lhsT=PTs[:], rhs=PM[:, C:])
    MT = pw.tile([D, C], f16, tag="MT", name=f"MT{c}_{s}")
    nc.vector.tensor_add(MT[:], PM[:, C:], Cp[:])

    # ---- D: solve U = MD(X + E MD(X + E MD(X + E MD X))) ----
    cur = X
    for it in range(3):
        Tp = pp.tile([D, 2 * C], f32, tag="pp", name=f"Tp{c}_{s}_{it}")
        nc.tensor.matmul(Tp[:], lhsT=MT[:], rhs=cur[:])
        Ts = pw.tile([D, 2 * C], f16, tag="Ts", name=f"Ts{c}_{s}_{it}")
        nc.scalar.copy(Ts[:], Tp[:])
        Ep = pp.tile([D, 2 * C], f32, tag="pp", name=f"Ep{c}_{s}_{it}")
        nc.tensor.matmul(Ep[:], lhsT=ETs[:], rhs=Ts[:])
        Xn = pw.tile([D, 2 * C], f16, tag="Xn", name=f"Xn{c}_{s}_{it}")
        nc.vector.tensor_add(Xn[:], X[:], Ep[:])
        cur = Xn
    Up = pp.tile([D, 2 * C], f32, tag="pp", name=f"Up{c}_{s}")
    nc.tensor.matmul(Up[:], lhsT=MT[:], rhs=cur[:])
    U = pw.tile([D, 2 * C], f16, tag="U", name=f"U{c}_{s}")
    nc.scalar.copy(U[:], Up[:])

    # ---- E: inter-chunk recurrence ----
    WTp = pp.tile([D, C], f32, tag="pp", name=f"WTp{c}_{s}")
    nc.tensor.matmul(WTp[:], lhsT=U[:, D:], rhs=ident[:])
    WTs = pw.tile([D, C], f16, tag="WTs", name=f"WTs{c}_{s}")
    nc.scalar.copy(WTs[:], WTp[:])

    if c == 0:
        up_ap = U[:, 0:D]
    else:
        upp = pp.tile([C, D], f32, tag="pp", name=f"upp{c}_{s}")
        nc.tensor.matmul(upp[:], lhsT=WTs[:], rhs=S_sb[s][:])
        ups = pw.tile([C, D], f16, tag="ups", name=f"ups{c}_{s}")
        nc.vector.tensor_sub(ups[:], U[:, 0:D], upp[:])
        up_ap = ups

    op_ = pp.tile([C, D], f32, tag="pp", name=f"op{c}_{s}")
    qT = RT[:, C:]
    if c == 0:
        nc.tensor.matmul(op_[:], lhsT=ALTs[:], rhs=up_ap[:])
    else:
        nc.tensor.matmul(op_[:], lhsT=qT, rhs=S_sb[s][:], start=True, stop=False)
        nc.tensor.matmul(op_[:], lhsT=ALTs[:], rhs=up_ap[:], start=False, stop=True)
    os_ = pw.tile([C, D], f32, tag="os", name=f"os{c}_{s}")
    nc.scalar.copy(os_[:], op_[:])
    nc.sync.dma_start(o_d[s, rows, :], os_[:])

    nc.tensor.matmul(S_ps[s][:], lhsT=kn[:], rhs=up_ap[:],
                     start=(c == 0), stop=(c == NCH - 1), skip_group_check=True)
    if c == NCH - 1:
        Sfin = pw.tile([D, D], f32, tag=f"Sfin{s}", name=f"Sfin{c}_{s}")
        nc.scalar.copy(Sfin[:], S_ps[s][:])
        nc.sync.dma_start(s_d[s], Sfin[:])
    else:
        Ssb = pw.tile([D, D], f16, tag=f"Ssb{s}", name=f"Ssb{c}_{s}")
        nc.scalar.copy(Ssb[:], S_ps[s][:])
        S_sb[s] = Ssb


def build(nch=NCH):
    global NCH
    NCH = nch
    ll = nch * C
    nc = bacc.Bacc("TRN2", target_bir_lowering=False, debug=False,
                   num_devices=N_CORES)
    q_d = nc.dram_tensor("q", (SPC, ll, D), f32, kind="ExternalInput").ap()
    k_d = nc.dram_tensor("k", (SPC, ll, D), f32, kind="ExternalInput").ap()
    v_d = nc.dram_tensor("v", (SPC, ll, D), f32, kind="ExternalInput").ap()
    beta_d = nc.dram_tensor("beta", (SPC, ll, 1), f32, kind="ExternalInput").ap()
    ident_d = nc.dram_tensor("ident", (D, D), f16, kind="ExternalInput").ap()
    mdt_d = nc.dram_tensor("mdt", (D, D), f32, kind="ExternalInput").ap()
    met_d = nc.dram_tensor("met", (D, D), f32, kind="ExternalInput").ap()
    mal_d = nc.dram_tensor("mal", (D, D), f32, kind="ExternalInput").ap()
    o_d = nc.dram_tensor("o", (SPC, ll, D), f32, kind="ExternalOutput").ap()
    s_d = nc.dram_tensor("s_out", (SPC, D, D), f32, kind="ExternalOutput").ap()

    with tile.TileContext(nc) as tc:
        with tc.tile_pool(name="consts", bufs=1) as pc, \
             tc.tile_pool(name="io", bufs=6) as pio, \
             tc.tile_pool(name="work", bufs=3) as pw, \
             tc.tile_pool(name="pp", bufs=6, space="PSUM") as pp, \
             tc.tile_pool(name="pS", bufs=1, space="PSUM") as pS:
            ident = pc.tile([D, D], f16, name="ident")
            mdt = pc.tile([D, D], f32, name="mdt")
            met = pc.tile([D, D], f32, name="met")
            mal = pc.tile([D, D], f32, name="mal")
            eps = pc.tile([D, 1], f32, name="eps")
            nc.vector.memset(eps[:], 1e-6)
            nc.sync.dma_start(ident[:], ident_d[:])
            nc.sync.dma_start(mdt[:], mdt_d[:])
            nc.sync.dma_start(met[:], met_d[:])
            nc.sync.dma_start(mal[:], mal_d[:])

            S_ps = [pS.tile([D, D], f32, name=f"Sps{s}") for s in range(SPC)]
            S_sb = [None] * SPC
            pools = (pio, pw, pp)
            consts = (ident, mdt, met, mal, eps, q_d, k_d, v_d, beta_d, o_d, s_d)
            state = (S_ps, S_sb)
            for c in range(nch):
                for s in range(SPC):
                    _emit_chunk(nc, pools, consts, state, c, s)
    nc.compile()
    return nc


def _consts():
    ii, jj = np.meshgrid(np.arange(C), np.arange(C), indexing="ij")
    bi, bj = ii // SUB, jj // SUB
    mdt = np.where((bi == bj) & (jj > ii), -1.0, 0.0).astype(np.float32)
    met = np.where(bj > bi, -1.0, 0.0).astype(np.float32)
    mal = np.where(jj >= ii, 1.0, 0.0).astype(np.float32)
    ident = np.eye(C, dtype=np.float16)
    return ident, mdt, met, mal


_NC_CACHE = {}


def kernel(q, k, v, beta, chunk_size=None):
    q = np.ascontiguousarray(np.asarray(q, dtype=np.float32))
    k = np.ascontiguousarray(np.asarray(k, dtype=np.float32))
    v = np.ascontiguousarray(np.asarray(v, dtype=np.float32))
    beta = np.ascontiguousarray(np.asarray(beta, dtype=np.float32))
    b, h, ll, d = q.shape
    assert (b, h, ll, d) == (B, H, L, D)

    if "nc" not in _NC_CACHE:
        _NC_CACHE["nc"] = build()
    nc = _NC_CACHE["nc"]

    ident, mdt, met, mal = _consts()
    qf = q.reshape(NSL, ll, d)
    kf = k.reshape(NSL, ll, d)
    vf = v.reshape(NSL, ll, d)
    bf = beta.reshape(NSL, ll, 1)
    in_maps = []
    for i in range(N_CORES):
        sl = slice(i * SPC, (i + 1) * SPC)
        in_maps.append({
            "q": qf[sl], "k": kf[sl], "v": vf[sl], "beta": bf[sl],
            "ident": ident, "mdt": mdt, "met": met, "mal": mal,
        })
    res = bass_utils.run_bass_kernel_spmd(nc, in_maps, core_ids=list(range(N_CORES)))
    o = np.concatenate([res.results[i]["o"] for i in range(N_CORES)], axis=0)
    S = np.concatenate([res.results[i]["s_out"] for i in range(N_CORES)], axis=0)
    return o.reshape(B, H, ll, d), S.reshape(B, H, d, d)
